# revision 26
# baseline (speedup 1.0000x reference)
"""Trainium2 Bass kernel for nn_ActionPredictionNet (GNN message passing).

Data-parallel over batch*particles: 8 NeuronCores, each handling 256
independent fully-connected 10-node particle graphs (2560 nodes, 23040
edges). The fully-connected structure lets us restructure the math:

  - Edge-MLP layer 1 collapses: e_in = [n[s], n[r]] so layer-1 pre-act is
    u[s] + v[r] with u = W_top^T n, v = W_bot^T n computed per NODE
    (2560 cols) instead of per EDGE (23040 cols), then a broadcast-add.
  - Edges are only consumed via the mean over incoming messages, so edge
    layer 3 folds into the aggregation: accumulate (sum_s h2_s) @ (w_e3/9)
    directly in PSUM (edge columns skip the s == r diagonal entirely).

Layouts (per core, feat-major: features on SBUF partitions):
  - node tensors [128, 2560], column = a*256 + p  (a: node-in-graph 0..9,
    p: graph 0..255)  -> broadcast APs get innermost unit stride.
  - edge tensors [128, 23040], column = r*2304 + s'*256 + p, where s' is
    the sender slot 0..8 (senders skip s == r).

Engine budget (per-trace measured rates): PE ~30us of matmul issue;
PSUM evictions ~0.83ns/col on ACT, ~1.16ns/col on DVE; h1 broadcast-adds
and relu passes are DVE-only SBUF fp16 work. The schedule splits the
~65-70us of ACT/DVE work evenly (~35us each) and pipelines the edge
stream against the node-MLP tail with a one-chunk lag so no engine
queue inverts. All matmuls fp16 x fp16 -> fp32 PSUM; the final logit
bias is added on the host (output is evicted as a raw fp32 copy).
"""

import numpy as np

B, P, A = 32, 64, 10
S_DIM, H_DIM, MID = 64, 64, 128
ACT = 8
N_CORES = 8
NP_CORE = B * P // N_CORES          # 256 particle-graphs per core
NODES = NP_CORE * A                 # 2560 nodes per core
QB = (A - 1) * NP_CORE              # 2304 edge columns per receiver block
ECOLS = A * QB                      # 23040 (r, s', p) edge columns per core
EG = 1024                           # h2 PSUM group width
NG = (ECOLS + EG - 1) // EG         # 23 groups (22x1024 + 512)

_PROG = None        # cached compiled program: (nc, meta)
LAST_EXEC_NS = None  # filled when KERNEL_TRACE=1


# ---------------------------------------------------------------- host utils

def _expected_edges():
    a = np.arange(A)
    s, r = np.meshgrid(a, a, indexing="ij")
    m = s != r
    s, r = s[m], r[m]
    offs = (np.arange(B * P) * A)[:, None]
    return (offs + s[None, :]).reshape(-1).astype(np.int64), \
           (offs + r[None, :]).reshape(-1).astype(np.int64)


def _to_ap_major(x_core):
    """[2560, D] in (p, a) node order -> [D, 2560] feat-major, (a, p) cols."""
    return np.ascontiguousarray(
        x_core.reshape(NP_CORE, A, -1).transpose(1, 0, 2).reshape(NODES, -1).T
    )


def _from_ap_major(out_core):
    """[ACT, 2560] feat-major (a, p) cols -> [2560, ACT] in (p, a) order."""
    return out_core.T.reshape(A, NP_CORE, ACT).transpose(1, 0, 2).reshape(NODES, ACT)


def _fallback_numpy(theta, s, i, senders, receivers,
                    w_in1, b_in1, w_in2, b_in2,
                    w_e1, b_e1, w_e2, b_e2, w_e3, b_e3,
                    w_n1, b_n1, w_n2, b_n2, w_n3, b_n3,
                    w_l1, b_l1, w_l2, b_l2):
    """fp32 numpy replica of the reference; used only if inputs deviate from
    the documented structure (non-fully-connected edges or non-constant i)."""
    N = B * P * A
    relu = lambda x: np.maximum(x, 0.0)
    x = np.concatenate([theta.reshape(N, H_DIM), s.reshape(N, S_DIM),
                        i.reshape(N, 1)], axis=-1).astype(np.float32)
    n = relu(x @ w_in1 + b_in1) @ w_in2 + b_in2
    e_in = np.concatenate([n[senders], n[receivers]], axis=-1)
    e = relu(e_in @ w_e1 + b_e1)
    e = relu(e @ w_e2 + b_e2)
    e = e @ w_e3 + b_e3
    agg = np.zeros((N, e.shape[1]), np.float32)
    np.add.at(agg, receivers, e)
    agg /= (A - 1)
    h = np.concatenate([n, agg], axis=-1)
    h = relu(h @ w_n1 + b_n1)
    h = relu(h @ w_n2 + b_n2)
    h = h @ w_n3 + b_n3
    out = relu(h @ w_l1 + b_l1) @ w_l2 + b_l2
    return out.reshape(B, P, A, ACT).astype(np.float32)


# ------------------------------------------------------------- device program

# weight-pack slot indices (all linear-linear layer pairs folded on host:
# w_in2 into we1t/we1b/wn1a, w_e3/9 into wn1b, w_n3 into w_l1)
W1T, WE1T, WE1B, WE2, WN1A, WN1B, WN2, WL1, WL2 = range(9)
NSLOTS = 9
# bias-pack column indices (BU carries both edge-layer-1 halves' biases and
# is applied in the h1 relu pass; the final logit bias is added on the host)
B1, BU, BE2, BN1, BN2, BL1 = range(6)

# h2 eviction groups routed to the vector engine (rest go to scalar/ACT);
# chosen to balance ACT ~= DVE busy time.
DVE_H2 = {6, 14, 16, 18, 20}


def _build_program():
    import concourse.bass as bass
    import concourse.mybir as mybir
    import concourse.tile as tile
    from concourse import bacc

    f16 = mybir.dt.float16
    f32 = mybir.dt.float32
    Af = mybir.ActivationFunctionType
    Op = mybir.AluOpType

    nc = bacc.Bacc("TRN2", target_bir_lowering=False, debug=False)
    x_dram = nc.dram_tensor("x_fm", [128, NODES], f16, kind="ExternalInput").ap()
    w_dram = nc.dram_tensor("w_pack", [128, NSLOTS * 128], f16,
                            kind="ExternalInput").ap()
    b_dram = nc.dram_tensor("b_pack", [128, 8], f32, kind="ExternalInput").ap()
    out_dram = nc.dram_tensor("out", [ACT, NODES], f32, kind="ExternalOutput").ap()

    with tile.TileContext(nc) as tc:
        with (
            tc.tile_pool(name="consts", bufs=1) as consts,
            tc.tile_pool(name="bigs", bufs=1) as bigs,
            tc.tile_pool(name="psA", bufs=2, space="PSUM") as psA,
            tc.tile_pool(name="psB", bufs=2, space="PSUM") as psB,
        ):
            wt = consts.tile([128, NSLOTS * 128], f16, tag="wt")
            bt = consts.tile([128, 8], f32, tag="bt")
            x_fm = bigs.tile([128, NODES], f16, tag="x_fm")
            dummy = consts.tile([128, 512], f16, tag="dummy")
            dsink = consts.tile([128, 8], f32, tag="dsink")

            # input DMAs: x in 3 chunks matching the node-layer groups so the
            # encoder starts on the first chunk; small/early tensors first,
            # the bulk weight pack last (only needed from the edge stream on).
            nc.scalar.dma_start(out=x_fm[:, 0:512], in_=x_dram[:, 0:512])
            nc.sync.dma_start(out=wt[:, :384], in_=w_dram[:, :384])
            nc.sync.dma_start(out=bt[:], in_=b_dram)
            nc.gpsimd.dma_start(out=x_fm[:, 512:1536], in_=x_dram[:, 512:1536])
            nc.gpsimd.dma_start(out=wt[:, 384:], in_=w_dram[:, 384:])
            nc.vector.memset(dummy[:], 0.0)
            # touch the Relu table set early so ACT_TABLE_LOAD hides in the
            # DMA-wait head instead of stalling the first eviction
            nc.scalar.activation(dsink[:, 0:1], dummy[:, 0:2].bitcast(f32),
                                 Af.Relu)
            nc.scalar.dma_start(out=x_fm[:, 1536:2560], in_=x_dram[:, 1536:2560])

            W = lambda k: wt[:, k * 128:(k + 1) * 128]
            bias = lambda k: bt[:, k:k + 1]

            # a few dummy matmuls while waiting on DMA: warms the PE HAM
            # activity window so the encoder runs at full clock.
            dfill = psA.tile([128, EG], f32, tag="psA")
            for _ in range(3):
                nc.tensor.matmul(dfill[:, :384], dummy[:, :128],
                                 dummy[:, :384], start=True, stop=True)
            nc.vector.tensor_copy(dsink[:], dfill[:, :8])

            t_enc = bigs.tile([128, NODES], f16, tag="t_enc")
            u_t = bigs.tile([128, NODES], f16, tag="u_t")
            v_t = bigs.tile([128, NODES], f16, tag="v_t")
            h1_t = bigs.tile([128, ECOLS], f16, tag="h1_t")
            h2_t = bigs.tile([128, ECOLS], f16, tag="h2_t")
            t_n1 = bigs.tile([128, NODES], f16, tag="t_n1")
            t_n2 = bigs.tile([128, NODES], f16, tag="t_n2")
            t_l1 = bigs.tile([128, NODES], f16, tag="t_l1")
            out_sb = bigs.tile([ACT, NODES], f32, tag="out_sb")

            h2r = h2_t[:].rearrange("f (r q) -> f r q", q=QB)
            v3 = v_t[:].rearrange("f (r p) -> f r p", p=NP_CORE)

            def mm512(ps, wk, src, c0, cw):
                for o in range(0, cw, 512):
                    w_ = min(512, cw - o)
                    nc.tensor.matmul(ps[:, o:o + w_], W(wk),
                                     src[:, c0 + o:c0 + o + w_],
                                     start=True, stop=True)

            # PSUM tiles alternate between the two pools (4 bufs in flight)
            _ti = [0]

            def ps_tile():
                _ti[0] += 1
                pool = psA if _ti[0] % 2 else psB
                return pool.tile([128, EG], f32, name="ps_nb",
                                 tag="psA" if _ti[0] % 2 else "psB")

            done = [0] * A          # h1 cols completed within receiver block
            state = {"next_h2": 0, "half_done": [False] * 6}
            slab_q = []             # pending decoder stages

            def h1_prefix():
                cols = 0
                for rr in range(A):
                    cols += done[rr]
                    if done[rr] < QB:
                        break
                return cols

            def emit_tt_part(r, a, b):
                """h1 adds+relu for receiver r restricted to u cols [a, b):
                broadcast-adds then fused (b_u + b_v)+relu, all on DVE."""
                w0 = r * QB
                vb1 = v3[:, r:r + 1, :]
                spans = ((max(a, 0), min(b, r * NP_CORE), 0),
                         (max(a, (r + 1) * NP_CORE), b, NP_CORE))
                d_lo, d_hi = None, None
                for lo, hi, sh in spans:
                    if hi <= lo:
                        continue
                    k = (hi - lo) // NP_CORE
                    d0 = lo - sh
                    d_lo = d0 if d_lo is None else d_lo
                    d_hi = d0 + k * NP_CORE
                    o = h1_t[:, w0 + d0:w0 + d_hi] \
                        .rearrange("f (s p) -> f s p", p=NP_CORE)
                    us = u_t[:, lo:hi] \
                        .rearrange("f (s p) -> f s p", p=NP_CORE)
                    nc.vector.tensor_add(o, us,
                                         vb1.broadcast_to([128, k, NP_CORE]))
                if d_lo is None:
                    return
                flat = h1_t[:, w0 + d_lo:w0 + d_hi]
                nc.vector.tensor_scalar(flat, flat, bias(BU), 0.0,
                                        Op.add, Op.max)
                done[r] = d_hi

            def emit_h2_group(g):
                g0 = g * EG
                gw = min(EG, ECOLS - g0)
                ps = psA.tile([128, EG], f32, tag="psA")
                mm512(ps, WE2, h1_t, g0, gw)
                if g in DVE_H2:
                    nc.vector.tensor_scalar(h2_t[:, g0:g0 + gw], ps[:, :gw],
                                            bias(BE2), 0.0, Op.add, Op.max)
                else:
                    nc.scalar.activation(h2_t[:, g0:g0 + gw], ps[:, :gw],
                                         Af.Relu, bias=bias(BE2))

            def emit_agg_n1(s0, sw, r_lo, eng="act"):
                """t_n1 cols [s0, s0+sw) = relu(wn1a^T t_enc
                + wn1b^T sum_s h2 + b) for receivers r_lo..r_lo+sw/256-1."""
                nr = sw // NP_CORE
                ps = psB.tile([128, EG], f32, tag="psB")
                nc.tensor.matmul(ps[:, :sw], W(WN1A), t_enc[:, s0:s0 + sw],
                                 start=True, stop=False)
                for s_ in range(A - 1):
                    nc.tensor.matmul(
                        ps[:, :sw], W(WN1B),
                        h2r[:, r_lo:r_lo + nr,
                            s_ * NP_CORE:(s_ + 1) * NP_CORE],
                        start=False, stop=(s_ == A - 2))
                if eng == "act":
                    nc.scalar.activation(t_n1[:, s0:s0 + sw], ps[:, :sw],
                                         Af.Relu, bias=bias(BN1))
                else:
                    nc.vector.tensor_scalar(t_n1[:, s0:s0 + sw], ps[:, :sw],
                                            bias(BN1), 0.0, Op.add, Op.max)

            def emit_stage(stage, s0, sw, eng):
                """One decoder stage for t_n1 cols [s0, s0+sw)."""
                ps = psB.tile([128, EG], f32, tag="psB")
                if stage == "n2":
                    mm512(ps, WN2, t_n1, s0, sw)
                    src, dst, bi = ps[:, :sw], t_n2[:, s0:s0 + sw], BN2
                elif stage == "l1":
                    mm512(ps, WL1, t_n2, s0, sw)
                    src, dst, bi = ps[:, :sw], t_l1[:, s0:s0 + sw], BL1
                else:  # "out": raw fp32 copy; logit bias is added on host
                    mm512(ps, WL2, t_l1, s0, sw)
                    if eng == "act":
                        nc.scalar.copy(out_sb[:, s0:s0 + sw], ps[:ACT, :sw])
                    else:
                        nc.vector.tensor_copy(out_sb[:, s0:s0 + sw],
                                              ps[:ACT, :sw])
                    nc.sync.dma_start(out=out_dram[:, s0:s0 + sw],
                                      in_=out_sb[:, s0:s0 + sw])
                    return
                if eng == "act":
                    nc.scalar.activation(dst, src, Af.Relu, bias=bias(bi))
                else:
                    nc.vector.tensor_scalar(dst, src, bias(bi), 0.0,
                                            Op.add, Op.max)

            # agg halves: four 512-col receiver pairs, then the last two
            # receivers separately (256 cols each) so the final decoder
            # chains start as soon as their own h2 blocks complete.
            halves = [(0, 512, 0), (512, 512, 2), (1024, 512, 4),
                      (1536, 512, 6), (2048, 256, 8), (2304, 256, 9)]
            half_gate = [-(-QB * (r_lo + sw // NP_CORE) // EG)
                         for (s0, sw, r_lo) in halves]

            def pump():
                cols = h1_prefix()
                while True:
                    g = state["next_h2"]
                    if g >= NG or min(ECOLS, (g + 1) * EG) > cols:
                        break
                    emit_h2_group(g)
                    state["next_h2"] += 1
                    if slab_q:
                        emit_stage(*slab_q.pop(0))
                    for h, (s0, sw, r_lo) in enumerate(halves):
                        if (not state["half_done"][h]
                                and state["next_h2"] >= half_gate[h]):
                            state["half_done"][h] = True
                            emit_agg_n1(s0, sw, r_lo,
                                        "act" if h != 5 else "vec")
                            if h == 1:
                                for st in ("n2", "l1", "out"):
                                    slab_q.append((st, 0, 1024, "act"))
                            elif h == 3:
                                for st, eng in (("n2", "vec"), ("l1", "act"),
                                                ("out", "vec")):
                                    slab_q.append((st, 1024, 1024, eng))
                            elif h == 4:
                                for st in ("n2", "l1", "out"):
                                    slab_q.append((st, 2048, 256, "act"))
                            elif h == 5:
                                # interleave the r9 chain between remaining
                                # r8 stages so the two tails run in parallel
                                tail_a = slab_q[:]
                                del slab_q[:]
                                tail_b = [(st, 2304, 256, "vec")
                                          for st in ("n2", "l1", "out")]
                                while tail_b or tail_a:
                                    if tail_b:
                                        slab_q.append(tail_b.pop(0))
                                    if tail_a:
                                        slab_q.append(tail_a.pop(0))

            # ---- node phase, interleaved per x-chunk: encoder (ACT relu
            # evict), u (ACT copy), v (DVE copy); h1 adds for receivers 0-1
            # start piecewise as u chunks land so the edge stream opens
            # early. No h2 groups are emitted inside this loop — that would
            # head-of-line-block later node matmuls in the PE queue — and
            # each chunk's adds precede its v-cast in the DVE queue (they
            # only need v block 0, produced by chunk 0).
            for ci, (c0, cw) in enumerate(((0, 512), (512, 1024),
                                           (1536, 1024))):
                ps = ps_tile()
                mm512(ps, W1T, x_fm, c0, cw)
                nc.scalar.activation(t_enc[:, c0:c0 + cw], ps[:, :cw],
                                     Af.Relu, bias=bias(B1))
                ps = ps_tile()
                mm512(ps, WE1T, t_enc, c0, cw)
                nc.vector.tensor_copy(u_t[:, c0:c0 + cw], ps[:, :cw])
                if ci > 0:
                    for r in (0, 1):
                        emit_tt_part(r, c0, c0 + cw)
                if ci == 2:
                    # the first h2 groups only need receiver-0/1 h1 pieces;
                    # emitting them before the last v keeps the PE dense
                    # through the node-to-edge transition
                    pump()
                ps = ps_tile()
                mm512(ps, WE1B, t_enc, c0, cw)
                nc.scalar.copy(v_t[:, c0:c0 + cw], ps[:, :cw])
                if ci == 0:
                    for r in (0, 1):
                        emit_tt_part(r, c0, c0 + cw)
            pump()

            for r in range(2, 8):
                emit_tt_part(r, 0, NODES)
                pump()
            for r in (8, 9):
                for a, b_ in ((0, 1024), (1024, 2048), (2048, NODES)):
                    emit_tt_part(r, a, b_)
                    pump()

            # keep the PE activity window warm across the decoder tail
            dfill2 = psA.tile([128, EG], f32, tag="psA")
            for _ in range(2):
                nc.tensor.matmul(dfill2[:, :512], dummy[:, :128],
                                 h1_t[:, ECOLS - 512:], start=True, stop=True)
            nc.vector.tensor_copy(dsink[:], dfill2[:, :8])

            while slab_q:
                emit_stage(*slab_q.pop(0))

    nc.compile()
    _dedupe_ldweights(nc)
    return nc


def _dedupe_ldweights(nc):
    """Remove redundant PE weight loads after bacc splits matmuls into
    Ldweights+Matmult pairs: a Ldweights whose source AP equals the
    previously loaded one (PE stream order == block order) is a no-op.
    Only drop instructions carrying no semaphore waits/updates."""
    from concourse import mybir
    import bass_rust
    for f in nc.m.functions:
        for b in f.blocks:
            last = None
            keep = []
            insts = b.instructions
            for idx, i in enumerate(insts):
                if isinstance(i, mybir.InstLdweights):
                    key = str(i.ins[0])
                    if key == last:
                        if i.sync_info is None:
                            continue
                        # migrate waits/updates onto the paired matmult so
                        # the redundant load can still be dropped
                        nxt = insts[idx + 1] if idx + 1 < len(insts) else None
                        if isinstance(nxt, mybir.InstMatmult):
                            ow = list(i.sync_info.on_wait)
                            ou = list(i.sync_info.on_update)
                            if nxt.sync_info is not None:
                                ow += list(nxt.sync_info.on_wait)
                                ou += list(nxt.sync_info.on_update)
                            if len(ow) <= 1:    # walrus: one wait per inst
                                nxt.sync_info = bass_rust.SyncInfo(
                                    on_wait=ow, on_update=ou)
                                continue
                    last = key
                keep.append(i)
            if len(keep) != len(insts):
                b.instructions[:] = keep


def _get_program():
    global _PROG
    if _PROG is None:
        _PROG = _build_program()
    return _PROG


# ------------------------------------------------------------------- kernel

def kernel(theta, s, i, senders, receivers,
           w_in1, b_in1, w_in2, b_in2,
           w_e1, b_e1, w_e2, b_e2, w_e3, b_e3,
           w_n1, b_n1, w_n2, b_n2, w_n3, b_n3,
           w_l1, b_l1, w_l2, b_l2):
    global LAST_EXEC_NS
    import os

    args = dict(theta=theta, s=s, i=i, senders=senders, receivers=receivers,
                w_in1=w_in1, b_in1=b_in1, w_in2=w_in2, b_in2=b_in2,
                w_e1=w_e1, b_e1=b_e1, w_e2=w_e2, b_e2=b_e2,
                w_e3=w_e3, b_e3=b_e3, w_n1=w_n1, b_n1=b_n1,
                w_n2=w_n2, b_n2=b_n2, w_n3=w_n3, b_n3=b_n3,
                w_l1=w_l1, b_l1=b_l1, w_l2=w_l2, b_l2=b_l2)
    args = {k: np.asarray(v) for k, v in args.items()}

    # The device program hardcodes the documented block-diagonal
    # fully-connected edge structure and constant-i input; verify, else
    # fall back to a host fp32 computation (correct for any input).
    exp_s, exp_r = _expected_edges()
    i_flat = np.asarray(args["i"], np.float32).reshape(-1)
    structured = (np.array_equal(np.asarray(args["senders"], np.int64), exp_s)
                  and np.array_equal(np.asarray(args["receivers"], np.int64), exp_r)
                  and np.all(i_flat == i_flat[0]))
    if not structured:
        return _fallback_numpy(**{k: np.asarray(v, np.float32)
                                  if np.asarray(v).dtype != np.int32 else np.asarray(v)
                                  for k, v in args.items()})

    f64 = np.float64
    w_in1_, b_in1_ = args["w_in1"].astype(f64), args["b_in1"].astype(f64)
    w_in2_, b_in2_ = args["w_in2"].astype(f64), args["b_in2"].astype(f64)
    w_e1_, b_e1_ = args["w_e1"].astype(f64), args["b_e1"].astype(f64)
    w_e3_, b_e3_ = args["w_e3"].astype(f64), args["b_e3"].astype(f64)
    w_n1_, b_n1_ = args["w_n1"].astype(f64), args["b_n1"].astype(f64)
    w_n3_, b_n3_ = args["w_n3"].astype(f64), args["b_n3"].astype(f64)
    w_l1_, b_l1_ = args["w_l1"].astype(f64), args["b_l1"].astype(f64)

    b1_eff = b_in1_ + i_flat[0] * w_in1_[H_DIM + S_DIM]
    # both edge-layer-1 bias halves ride the h1 relu pass
    b_u = b_e1_ + b_in2_ @ w_e1_[:MID] + b_in2_ @ w_e1_[MID:]
    b_n1_eff = b_n1_ + b_in2_ @ w_n1_[:MID] + b_e3_ @ w_n1_[MID:]
    b_l1_eff = b_l1_ + b_n3_ @ w_l1_

    wslots = np.zeros((NSLOTS, 128, 128), np.float16)
    wslots[W1T] = w_in1_[:128].astype(np.float16)
    wslots[WE1T] = (w_in2_ @ w_e1_[:MID]).astype(np.float16)
    wslots[WE1B] = (w_in2_ @ w_e1_[MID:]).astype(np.float16)
    wslots[WE2] = args["w_e2"].astype(np.float16)
    wslots[WN1A] = (w_in2_ @ w_n1_[:MID]).astype(np.float16)
    wslots[WN1B] = ((w_e3_ / (A - 1)) @ w_n1_[MID:]).astype(np.float16)
    wslots[WN2] = args["w_n2"].astype(np.float16)
    wslots[WL1] = (w_n3_ @ w_l1_).astype(np.float16)
    wslots[WL2, :, :ACT] = args["w_l2"].astype(np.float16)
    w_pack = np.ascontiguousarray(
        wslots.transpose(1, 0, 2).reshape(128, NSLOTS * 128))

    b_pack = np.zeros((128, 8), np.float32)
    for idx, vec in ((B1, b1_eff), (BU, b_u), (BE2, args["b_e2"]),
                     (BN1, b_n1_eff), (BN2, args["b_n2"]), (BL1, b_l1_eff)):
        b_pack[:, idx] = np.asarray(vec, np.float32)
    b_l2_host = args["b_l2"].astype(np.float32)

    # node features, feat-major, (a, p) column order, per-core shards
    n_all = B * P * A
    X = np.concatenate([args["theta"].reshape(n_all, H_DIM),
                        args["s"].reshape(n_all, S_DIM)], axis=-1)
    in_maps = []
    for c in range(N_CORES):
        xc = X[c * NODES:(c + 1) * NODES]
        in_maps.append({
            "x_fm": _to_ap_major(xc).astype(np.float16),
            "w_pack": w_pack,
            "b_pack": b_pack,
        })

    nc = _get_program()
    if os.environ.get("KERNEL_SIM", "0") == "1":
        # CoreSim core 0 only (cores are identical up to data); other cores
        # return zeros. For correctness devloop, not grading.
        from concourse import bass_interp
        sim = bass_interp.CoreSim(nc)
        for k, v in in_maps[0].items():
            sim.tensor(k)[:] = v
        sim.simulate()
        results = [{"out": np.array(sim.tensor("out"))}]
        results += [{"out": np.zeros((ACT, NODES), np.float32)}
                    for _ in range(N_CORES - 1)]
        parts = [_from_ap_major(r["out"]) for r in results]
        out = np.concatenate(parts, axis=0).reshape(B, P, A, ACT)
        out[:4] += b_l2_host
        return out.astype(np.float32)

    from concourse.bass_utils import run_bass_kernel_spmd
    trace = os.environ.get("KERNEL_TRACE", "0") == "1"
    res = run_bass_kernel_spmd(nc, in_maps, core_ids=list(range(N_CORES)),
                               trace=trace)
    LAST_EXEC_NS = res.exec_time_ns

    parts = [_from_ap_major(res.results[c]["out"]) for c in range(N_CORES)]
    out = np.concatenate(parts, axis=0).reshape(B, P, A, ACT) + b_l2_host
    return out.astype(np.float32)


# revision 27
# speedup vs baseline: 1.0394x; 1.0394x over previous
"""Trainium2 Bass kernel for nn_ActionPredictionNet (GNN message passing).

Data-parallel over batch*particles: 8 NeuronCores, each handling 256
independent fully-connected 10-node particle graphs (2560 nodes, 23040
edges). The fully-connected structure lets us restructure the math:

  - Edge-MLP layer 1 collapses: e_in = [n[s], n[r]] so layer-1 pre-act is
    u[s] + v[r] with u = W_top^T n, v = W_bot^T n computed per NODE
    (2560 cols) instead of per EDGE (23040 cols), then a broadcast-add.
  - Edges are only consumed via the mean over incoming messages, so edge
    layer 3 folds into the aggregation: accumulate (sum_s h2_s) @ (w_e3/9)
    directly in PSUM (edge columns skip the s == r diagonal entirely).

Layouts (per core, feat-major: features on SBUF partitions):
  - node tensors [128, 2560], column = a*256 + p  (a: node-in-graph 0..9,
    p: graph 0..255)  -> broadcast APs get innermost unit stride.
  - edge tensors [128, 23040], column = r*2304 + s'*256 + p, where s' is
    the sender slot 0..8 (senders skip s == r).

Engine budget (per-trace measured rates): PE ~30us of matmul issue;
PSUM evictions ~0.83ns/col on ACT, ~1.16ns/col on DVE; h1 broadcast-adds
and relu passes are DVE-only SBUF fp16 work. The schedule splits the
~65-70us of ACT/DVE work evenly (~35us each) and pipelines the edge
stream against the node-MLP tail with a one-chunk lag so no engine
queue inverts. All matmuls fp16 x fp16 -> fp32 PSUM; the final logit
bias is added on the host (output is evicted as a raw fp32 copy).
"""

import numpy as np

B, P, A = 32, 64, 10
S_DIM, H_DIM, MID = 64, 64, 128
ACT = 8
N_CORES = 8
NP_CORE = B * P // N_CORES          # 256 particle-graphs per core
NODES = NP_CORE * A                 # 2560 nodes per core
QB = (A - 1) * NP_CORE              # 2304 edge columns per receiver block
ECOLS = A * QB                      # 23040 (r, s', p) edge columns per core
EG = 1024                           # h2 PSUM group width
NG = (ECOLS + EG - 1) // EG         # 23 groups (22x1024 + 512)

_PROG = None        # cached compiled program: (nc, meta)
LAST_EXEC_NS = None  # filled when KERNEL_TRACE=1


# ---------------------------------------------------------------- host utils

def _expected_edges():
    a = np.arange(A)
    s, r = np.meshgrid(a, a, indexing="ij")
    m = s != r
    s, r = s[m], r[m]
    offs = (np.arange(B * P) * A)[:, None]
    return (offs + s[None, :]).reshape(-1).astype(np.int64), \
           (offs + r[None, :]).reshape(-1).astype(np.int64)


def _to_ap_major(x_core):
    """[2560, D] in (p, a) node order -> [D, 2560] feat-major, (a, p) cols."""
    return np.ascontiguousarray(
        x_core.reshape(NP_CORE, A, -1).transpose(1, 0, 2).reshape(NODES, -1).T
    )


def _from_ap_major(out_core):
    """[ACT, 2560] feat-major (a, p) cols -> [2560, ACT] in (p, a) order."""
    return out_core.T.reshape(A, NP_CORE, ACT).transpose(1, 0, 2).reshape(NODES, ACT)


def _fallback_numpy(theta, s, i, senders, receivers,
                    w_in1, b_in1, w_in2, b_in2,
                    w_e1, b_e1, w_e2, b_e2, w_e3, b_e3,
                    w_n1, b_n1, w_n2, b_n2, w_n3, b_n3,
                    w_l1, b_l1, w_l2, b_l2):
    """fp32 numpy replica of the reference; used only if inputs deviate from
    the documented structure (non-fully-connected edges or non-constant i)."""
    N = B * P * A
    relu = lambda x: np.maximum(x, 0.0)
    x = np.concatenate([theta.reshape(N, H_DIM), s.reshape(N, S_DIM),
                        i.reshape(N, 1)], axis=-1).astype(np.float32)
    n = relu(x @ w_in1 + b_in1) @ w_in2 + b_in2
    e_in = np.concatenate([n[senders], n[receivers]], axis=-1)
    e = relu(e_in @ w_e1 + b_e1)
    e = relu(e @ w_e2 + b_e2)
    e = e @ w_e3 + b_e3
    agg = np.zeros((N, e.shape[1]), np.float32)
    np.add.at(agg, receivers, e)
    agg /= (A - 1)
    h = np.concatenate([n, agg], axis=-1)
    h = relu(h @ w_n1 + b_n1)
    h = relu(h @ w_n2 + b_n2)
    h = h @ w_n3 + b_n3
    out = relu(h @ w_l1 + b_l1) @ w_l2 + b_l2
    return out.reshape(B, P, A, ACT).astype(np.float32)


# ------------------------------------------------------------- device program

# weight-pack slot indices (all linear-linear layer pairs folded on host:
# w_in2 into we1t/we1b/wn1a, w_e3/9 into wn1b, w_n3 into w_l1)
W1T, WE1T, WE1B, WE2, WN1A, WN1B, WN2, WL1, WL2 = range(9)
NSLOTS = 9
# bias-pack column indices (BU carries both edge-layer-1 halves' biases and
# is applied in the h1 relu pass; the final logit bias is added on the host)
B1, BU, BE2, BN1, BN2, BL1 = range(6)

# h2 eviction groups routed to the vector engine (rest go to scalar/ACT);
# chosen to balance ACT ~= DVE busy time.
DVE_H2 = {6, 9, 11, 14, 16, 18, 20}


def _build_program():
    import concourse.bass as bass
    import concourse.mybir as mybir
    import concourse.tile as tile
    from concourse import bacc

    f16 = mybir.dt.float16
    f32 = mybir.dt.float32
    Af = mybir.ActivationFunctionType
    Op = mybir.AluOpType

    nc = bacc.Bacc("TRN2", target_bir_lowering=False, debug=False)
    x_dram = nc.dram_tensor("x_fm", [128, NODES], f16, kind="ExternalInput").ap()
    w_dram = nc.dram_tensor("w_pack", [128, NSLOTS * 128], f16,
                            kind="ExternalInput").ap()
    b_dram = nc.dram_tensor("b_pack", [128, 8], f32, kind="ExternalInput").ap()
    out_dram = nc.dram_tensor("out", [ACT, NODES], f32, kind="ExternalOutput").ap()

    with tile.TileContext(nc) as tc:
        with (
            tc.tile_pool(name="consts", bufs=1) as consts,
            tc.tile_pool(name="bigs", bufs=1) as bigs,
            tc.tile_pool(name="psA", bufs=2, space="PSUM") as psA,
            tc.tile_pool(name="psB", bufs=2, space="PSUM") as psB,
        ):
            wt = consts.tile([128, NSLOTS * 128], f16, tag="wt")
            bt = consts.tile([128, 8], f32, tag="bt")
            x_fm = bigs.tile([128, NODES], f16, tag="x_fm")
            dummy = consts.tile([128, 512], f16, tag="dummy")
            dsink = consts.tile([128, 8], f32, tag="dsink")

            # input DMAs: x in 3 chunks matching the node-layer groups so the
            # encoder starts on the first chunk; small/early tensors first,
            # the bulk weight pack last (only needed from the edge stream on).
            nc.scalar.dma_start(out=x_fm[:, 0:512], in_=x_dram[:, 0:512])
            nc.sync.dma_start(out=wt[:, :384], in_=w_dram[:, :384])
            nc.sync.dma_start(out=bt[:], in_=b_dram)
            nc.gpsimd.dma_start(out=x_fm[:, 512:1536], in_=x_dram[:, 512:1536])
            nc.gpsimd.dma_start(out=wt[:, 384:], in_=w_dram[:, 384:])
            nc.vector.memset(dummy[:], 0.0)
            # touch the Relu table set early so ACT_TABLE_LOAD hides in the
            # DMA-wait head instead of stalling the first eviction
            nc.scalar.activation(dsink[:, 0:1], dummy[:, 0:2].bitcast(f32),
                                 Af.Relu)
            nc.scalar.dma_start(out=x_fm[:, 1536:2560], in_=x_dram[:, 1536:2560])

            W = lambda k: wt[:, k * 128:(k + 1) * 128]
            bias = lambda k: bt[:, k:k + 1]

            # a few dummy matmuls while waiting on DMA: warms the PE HAM
            # activity window so the encoder runs at full clock.
            dfill = psA.tile([128, EG], f32, tag="psA")
            for _ in range(3):
                nc.tensor.matmul(dfill[:, :384], dummy[:, :128],
                                 dummy[:, :384], start=True, stop=True)
            nc.vector.tensor_copy(dsink[:], dfill[:, :8])

            t_enc = bigs.tile([128, NODES], f16, tag="t_enc")
            u_t = bigs.tile([128, NODES], f16, tag="u_t")
            v_t = bigs.tile([128, NODES], f16, tag="v_t")
            h1_t = bigs.tile([128, ECOLS], f16, tag="h1_t")
            h2_t = bigs.tile([128, ECOLS], f16, tag="h2_t")
            t_n1 = bigs.tile([128, NODES], f16, tag="t_n1")
            t_n2 = bigs.tile([128, NODES], f16, tag="t_n2")
            t_l1 = bigs.tile([128, NODES], f16, tag="t_l1")
            out_sb = bigs.tile([ACT, NODES], f32, tag="out_sb")

            h2r = h2_t[:].rearrange("f (r q) -> f r q", q=QB)
            v3 = v_t[:].rearrange("f (r p) -> f r p", p=NP_CORE)

            def mm512(ps, wk, src, c0, cw):
                for o in range(0, cw, 512):
                    w_ = min(512, cw - o)
                    nc.tensor.matmul(ps[:, o:o + w_], W(wk),
                                     src[:, c0 + o:c0 + o + w_],
                                     start=True, stop=True)

            # PSUM tiles alternate between the two pools (4 bufs in flight)
            _ti = [0]

            def ps_tile():
                _ti[0] += 1
                pool = psA if _ti[0] % 2 else psB
                return pool.tile([128, EG], f32, name="ps_nb",
                                 tag="psA" if _ti[0] % 2 else "psB")

            done = [0] * A          # h1 cols completed within receiver block
            state = {"next_h2": 0, "half_done": [False] * 6}
            slab_q = []             # pending decoder stages

            def h1_prefix():
                cols = 0
                for rr in range(A):
                    cols += done[rr]
                    if done[rr] < QB:
                        break
                return cols

            def emit_tt_part(r, a, b):
                """h1 adds+relu for receiver r restricted to u cols [a, b):
                broadcast-adds then fused (b_u + b_v)+relu, all on DVE."""
                w0 = r * QB
                vb1 = v3[:, r:r + 1, :]
                spans = ((max(a, 0), min(b, r * NP_CORE), 0),
                         (max(a, (r + 1) * NP_CORE), b, NP_CORE))
                d_lo, d_hi = None, None
                for lo, hi, sh in spans:
                    if hi <= lo:
                        continue
                    k = (hi - lo) // NP_CORE
                    d0 = lo - sh
                    d_lo = d0 if d_lo is None else d_lo
                    d_hi = d0 + k * NP_CORE
                    o = h1_t[:, w0 + d0:w0 + d_hi] \
                        .rearrange("f (s p) -> f s p", p=NP_CORE)
                    us = u_t[:, lo:hi] \
                        .rearrange("f (s p) -> f s p", p=NP_CORE)
                    nc.vector.tensor_add(o, us,
                                         vb1.broadcast_to([128, k, NP_CORE]))
                if d_lo is None:
                    return
                flat = h1_t[:, w0 + d_lo:w0 + d_hi]
                nc.vector.tensor_scalar(flat, flat, bias(BU), 0.0,
                                        Op.add, Op.max)
                done[r] = d_hi

            def emit_h2_group(g):
                g0 = g * EG
                gw = min(EG, ECOLS - g0)
                ps = psA.tile([128, EG], f32, tag="psA")
                mm512(ps, WE2, h1_t, g0, gw)
                if g in DVE_H2:
                    nc.vector.tensor_scalar(h2_t[:, g0:g0 + gw], ps[:, :gw],
                                            bias(BE2), 0.0, Op.add, Op.max)
                else:
                    nc.scalar.activation(h2_t[:, g0:g0 + gw], ps[:, :gw],
                                         Af.Relu, bias=bias(BE2))

            def emit_agg_n1(s0, sw, r_lo, eng="act"):
                """t_n1 cols [s0, s0+sw) = relu(wn1a^T t_enc
                + wn1b^T sum_s h2 + b) for receivers r_lo..r_lo+sw/256-1."""
                nr = sw // NP_CORE
                ps = psB.tile([128, EG], f32, tag="psB")
                nc.tensor.matmul(ps[:, :sw], W(WN1A), t_enc[:, s0:s0 + sw],
                                 start=True, stop=False)
                for s_ in range(A - 1):
                    nc.tensor.matmul(
                        ps[:, :sw], W(WN1B),
                        h2r[:, r_lo:r_lo + nr,
                            s_ * NP_CORE:(s_ + 1) * NP_CORE],
                        start=False, stop=(s_ == A - 2))
                if eng == "act":
                    nc.scalar.activation(t_n1[:, s0:s0 + sw], ps[:, :sw],
                                         Af.Relu, bias=bias(BN1))
                else:
                    nc.vector.tensor_scalar(t_n1[:, s0:s0 + sw], ps[:, :sw],
                                            bias(BN1), 0.0, Op.add, Op.max)

            def emit_stage(stage, s0, sw, eng):
                """One decoder stage for t_n1 cols [s0, s0+sw)."""
                ps = psB.tile([128, EG], f32, tag="psB")
                if stage == "n2":
                    mm512(ps, WN2, t_n1, s0, sw)
                    src, dst, bi = ps[:, :sw], t_n2[:, s0:s0 + sw], BN2
                elif stage == "l1":
                    mm512(ps, WL1, t_n2, s0, sw)
                    src, dst, bi = ps[:, :sw], t_l1[:, s0:s0 + sw], BL1
                else:  # "out": raw fp32 copy; logit bias is added on host
                    mm512(ps, WL2, t_l1, s0, sw)
                    if eng == "act":
                        nc.scalar.copy(out_sb[:, s0:s0 + sw], ps[:ACT, :sw])
                    else:
                        nc.vector.tensor_copy(out_sb[:, s0:s0 + sw],
                                              ps[:ACT, :sw])
                    nc.sync.dma_start(out=out_dram[:, s0:s0 + sw],
                                      in_=out_sb[:, s0:s0 + sw])
                    return
                if eng == "act":
                    nc.scalar.activation(dst, src, Af.Relu, bias=bias(bi))
                else:
                    nc.vector.tensor_scalar(dst, src, bias(bi), 0.0,
                                            Op.add, Op.max)

            # agg halves: four 512-col receiver pairs, then the last two
            # receivers separately (256 cols each) so the final decoder
            # chains start as soon as their own h2 blocks complete.
            halves = [(0, 512, 0), (512, 512, 2), (1024, 512, 4),
                      (1536, 512, 6), (2048, 256, 8), (2304, 256, 9)]
            half_gate = [-(-QB * (r_lo + sw // NP_CORE) // EG)
                         for (s0, sw, r_lo) in halves]

            def pump():
                cols = h1_prefix()
                while True:
                    g = state["next_h2"]
                    if g >= NG or min(ECOLS, (g + 1) * EG) > cols:
                        break
                    emit_h2_group(g)
                    state["next_h2"] += 1
                    if slab_q:
                        emit_stage(*slab_q.pop(0))
                    for h, (s0, sw, r_lo) in enumerate(halves):
                        if (not state["half_done"][h]
                                and state["next_h2"] >= half_gate[h]):
                            state["half_done"][h] = True
                            emit_agg_n1(s0, sw, r_lo)
                            if h == 1:
                                for st in ("n2", "l1", "out"):
                                    slab_q.append((st, 0, 1024, "act"))
                            elif h == 3:
                                for st in ("n2", "l1", "out"):
                                    slab_q.append((st, 1024, 1024, "act"))
                            elif h == 4:
                                for st in ("n2", "l1", "out"):
                                    slab_q.append((st, 2048, 256, "act"))
                            elif h == 5:
                                # interleave the r9 chain between remaining
                                # r8 stages so the two tails run in parallel
                                tail_a = slab_q[:]
                                del slab_q[:]
                                tail_b = [(st, 2304, 256, "vec")
                                          for st in ("n2", "l1", "out")]
                                while tail_b or tail_a:
                                    if tail_b:
                                        slab_q.append(tail_b.pop(0))
                                    if tail_a:
                                        slab_q.append(tail_a.pop(0))

            # ---- node phase, interleaved per x-chunk: encoder (ACT relu
            # evict), u (ACT copy), v (DVE copy); h1 adds for receivers 0-1
            # start piecewise as u chunks land so the edge stream opens
            # early. No h2 groups are emitted inside this loop — that would
            # head-of-line-block later node matmuls in the PE queue — and
            # each chunk's adds precede its v-cast in the DVE queue (they
            # only need v block 0, produced by chunk 0).
            for ci, (c0, cw) in enumerate(((0, 512), (512, 1024),
                                           (1536, 1024))):
                ps = ps_tile()
                mm512(ps, W1T, x_fm, c0, cw)
                nc.scalar.activation(t_enc[:, c0:c0 + cw], ps[:, :cw],
                                     Af.Relu, bias=bias(B1))
                ps = ps_tile()
                mm512(ps, WE1T, t_enc, c0, cw)
                nc.vector.tensor_copy(u_t[:, c0:c0 + cw], ps[:, :cw])
                if ci > 0:
                    for r in (0, 1):
                        emit_tt_part(r, c0, c0 + cw)
                ps = ps_tile()
                mm512(ps, WE1B, t_enc, c0, cw)
                nc.scalar.copy(v_t[:, c0:c0 + cw], ps[:, :cw])
                if ci == 0:
                    for r in (0, 1):
                        emit_tt_part(r, c0, c0 + cw)
            pump()

            for r in range(2, 8):
                emit_tt_part(r, 0, NODES)
                pump()
            for r in (8, 9):
                for a, b_ in ((0, 1024), (1024, 2048), (2048, NODES)):
                    emit_tt_part(r, a, b_)
                    pump()

            # keep the PE activity window warm across the decoder tail
            dfill2 = psA.tile([128, EG], f32, tag="psA")
            for _ in range(2):
                nc.tensor.matmul(dfill2[:, :512], dummy[:, :128],
                                 h1_t[:, ECOLS - 512:], start=True, stop=True)
            nc.vector.tensor_copy(dsink[:], dfill2[:, :8])

            while slab_q:
                emit_stage(*slab_q.pop(0))

    nc.compile()
    _dedupe_ldweights(nc)
    return nc


def _dedupe_ldweights(nc):
    """Remove redundant PE weight loads after bacc splits matmuls into
    Ldweights+Matmult pairs: a Ldweights whose source AP equals the
    previously loaded one (PE stream order == block order) is a no-op.
    Only drop instructions carrying no semaphore waits/updates."""
    from concourse import mybir
    import bass_rust
    for f in nc.m.functions:
        for b in f.blocks:
            last = None
            keep = []
            insts = b.instructions
            for idx, i in enumerate(insts):
                if isinstance(i, mybir.InstLdweights):
                    key = str(i.ins[0])
                    if key == last:
                        if i.sync_info is None:
                            continue
                        # migrate waits/updates onto the paired matmult so
                        # the redundant load can still be dropped
                        nxt = insts[idx + 1] if idx + 1 < len(insts) else None
                        if isinstance(nxt, mybir.InstMatmult):
                            ow = list(i.sync_info.on_wait)
                            ou = list(i.sync_info.on_update)
                            if nxt.sync_info is not None:
                                ow += list(nxt.sync_info.on_wait)
                                ou += list(nxt.sync_info.on_update)
                            if len(ow) <= 1:    # walrus: one wait per inst
                                nxt.sync_info = bass_rust.SyncInfo(
                                    on_wait=ow, on_update=ou)
                                continue
                    last = key
                keep.append(i)
            if len(keep) != len(insts):
                b.instructions[:] = keep


def _get_program():
    global _PROG
    if _PROG is None:
        _PROG = _build_program()
    return _PROG


# ------------------------------------------------------------------- kernel

def kernel(theta, s, i, senders, receivers,
           w_in1, b_in1, w_in2, b_in2,
           w_e1, b_e1, w_e2, b_e2, w_e3, b_e3,
           w_n1, b_n1, w_n2, b_n2, w_n3, b_n3,
           w_l1, b_l1, w_l2, b_l2):
    global LAST_EXEC_NS
    import os

    args = dict(theta=theta, s=s, i=i, senders=senders, receivers=receivers,
                w_in1=w_in1, b_in1=b_in1, w_in2=w_in2, b_in2=b_in2,
                w_e1=w_e1, b_e1=b_e1, w_e2=w_e2, b_e2=b_e2,
                w_e3=w_e3, b_e3=b_e3, w_n1=w_n1, b_n1=b_n1,
                w_n2=w_n2, b_n2=b_n2, w_n3=w_n3, b_n3=b_n3,
                w_l1=w_l1, b_l1=b_l1, w_l2=w_l2, b_l2=b_l2)
    args = {k: np.asarray(v) for k, v in args.items()}

    # The device program hardcodes the documented block-diagonal
    # fully-connected edge structure and constant-i input; verify, else
    # fall back to a host fp32 computation (correct for any input).
    exp_s, exp_r = _expected_edges()
    i_flat = np.asarray(args["i"], np.float32).reshape(-1)
    structured = (np.array_equal(np.asarray(args["senders"], np.int64), exp_s)
                  and np.array_equal(np.asarray(args["receivers"], np.int64), exp_r)
                  and np.all(i_flat == i_flat[0]))
    if not structured:
        return _fallback_numpy(**{k: np.asarray(v, np.float32)
                                  if np.asarray(v).dtype != np.int32 else np.asarray(v)
                                  for k, v in args.items()})

    f64 = np.float64
    w_in1_, b_in1_ = args["w_in1"].astype(f64), args["b_in1"].astype(f64)
    w_in2_, b_in2_ = args["w_in2"].astype(f64), args["b_in2"].astype(f64)
    w_e1_, b_e1_ = args["w_e1"].astype(f64), args["b_e1"].astype(f64)
    w_e3_, b_e3_ = args["w_e3"].astype(f64), args["b_e3"].astype(f64)
    w_n1_, b_n1_ = args["w_n1"].astype(f64), args["b_n1"].astype(f64)
    w_n3_, b_n3_ = args["w_n3"].astype(f64), args["b_n3"].astype(f64)
    w_l1_, b_l1_ = args["w_l1"].astype(f64), args["b_l1"].astype(f64)

    b1_eff = b_in1_ + i_flat[0] * w_in1_[H_DIM + S_DIM]
    # both edge-layer-1 bias halves ride the h1 relu pass
    b_u = b_e1_ + b_in2_ @ w_e1_[:MID] + b_in2_ @ w_e1_[MID:]
    b_n1_eff = b_n1_ + b_in2_ @ w_n1_[:MID] + b_e3_ @ w_n1_[MID:]
    b_l1_eff = b_l1_ + b_n3_ @ w_l1_

    wslots = np.zeros((NSLOTS, 128, 128), np.float16)
    wslots[W1T] = w_in1_[:128].astype(np.float16)
    wslots[WE1T] = (w_in2_ @ w_e1_[:MID]).astype(np.float16)
    wslots[WE1B] = (w_in2_ @ w_e1_[MID:]).astype(np.float16)
    wslots[WE2] = args["w_e2"].astype(np.float16)
    wslots[WN1A] = (w_in2_ @ w_n1_[:MID]).astype(np.float16)
    wslots[WN1B] = ((w_e3_ / (A - 1)) @ w_n1_[MID:]).astype(np.float16)
    wslots[WN2] = args["w_n2"].astype(np.float16)
    wslots[WL1] = (w_n3_ @ w_l1_).astype(np.float16)
    wslots[WL2, :, :ACT] = args["w_l2"].astype(np.float16)
    w_pack = np.ascontiguousarray(
        wslots.transpose(1, 0, 2).reshape(128, NSLOTS * 128))

    b_pack = np.zeros((128, 8), np.float32)
    for idx, vec in ((B1, b1_eff), (BU, b_u), (BE2, args["b_e2"]),
                     (BN1, b_n1_eff), (BN2, args["b_n2"]), (BL1, b_l1_eff)):
        b_pack[:, idx] = np.asarray(vec, np.float32)
    b_l2_host = args["b_l2"].astype(np.float32)

    # node features, feat-major, (a, p) column order, per-core shards
    n_all = B * P * A
    X = np.concatenate([args["theta"].reshape(n_all, H_DIM),
                        args["s"].reshape(n_all, S_DIM)], axis=-1)
    in_maps = []
    for c in range(N_CORES):
        xc = X[c * NODES:(c + 1) * NODES]
        in_maps.append({
            "x_fm": _to_ap_major(xc).astype(np.float16),
            "w_pack": w_pack,
            "b_pack": b_pack,
        })

    nc = _get_program()
    if os.environ.get("KERNEL_SIM", "0") == "1":
        # CoreSim core 0 only (cores are identical up to data); other cores
        # return zeros. For correctness devloop, not grading.
        from concourse import bass_interp
        sim = bass_interp.CoreSim(nc)
        for k, v in in_maps[0].items():
            sim.tensor(k)[:] = v
        sim.simulate()
        results = [{"out": np.array(sim.tensor("out"))}]
        results += [{"out": np.zeros((ACT, NODES), np.float32)}
                    for _ in range(N_CORES - 1)]
        parts = [_from_ap_major(r["out"]) for r in results]
        out = np.concatenate(parts, axis=0).reshape(B, P, A, ACT)
        out[:4] += b_l2_host
        return out.astype(np.float32)

    from concourse.bass_utils import run_bass_kernel_spmd
    trace = os.environ.get("KERNEL_TRACE", "0") == "1"
    res = run_bass_kernel_spmd(nc, in_maps, core_ids=list(range(N_CORES)),
                               trace=trace)
    LAST_EXEC_NS = res.exec_time_ns

    parts = [_from_ap_major(res.results[c]["out"]) for c in range(N_CORES)]
    out = np.concatenate(parts, axis=0).reshape(B, P, A, ACT) + b_l2_host
    return out.astype(np.float32)
